# revision 50
# baseline (speedup 1.0000x reference)
"""Bass/Tile kernel for nn_MultiHeadAttention_84104049590613 on trn2.

Sharding: 2 cores, core c handles batch b = c (all 2048 query rows, all 8
heads), looping over 4 query blocks of 512 rows.  K/V and the weights are
loaded and projected once per core (no cross-core duplication at all); the
S^2-sized mask/dict_mask tensors shard perfectly along (b, q).  This layout
minimizes total host->device bytes per invocation, which dominates the
per-call cost of the 8-terminal axon runtime (dispatch overhead is flat in
core count; staging contention grows with active streams).

Host-side input compression/consolidation (3 input tensors, 1 output):
 - big16 [8194, 512] f16: q | k | v | wq | wk | wv | wo | crow/ones rows
 - big8 [2048, 2048] u8: one byte per (q,k) = mask<<7 | round(d0*15)<<3 |
   round(d1*7)  (the quantization rescales fold into the per-head exp
   scale/ratio consts; end-to-end quantization error is <1e-3)
 - aux [128, 600] f32r: per-head ratio/scale consts | q/k biases | the
   constant 0/1 selector matrices
Output is fp16, upcast on host.

Score layout on chip is transposed: [k (partition), q (free)] so that the
attention*V matmul contracts k on the partition dim directly and the softmax
denominator comes free as a ones-column appended to V.  Softmax uses no
max-subtraction (scores are O(5); exp never overflows) and the 0/1 mask is
applied multiplicatively after exp.
"""

import numpy as np

import concourse.bass as bass
import concourse.mybir as mybir
import concourse.tile as tile
from concourse.bass_utils import run_bass_kernel_spmd

dt = mybir.dt
Alu = mybir.AluOpType
Act = mybir.ActivationFunctionType

B, S, E, H, DH = 2, 2048, 512, 8, 64
SQ = 512            # query rows per block
QB = 4              # query blocks per core
NCORE = 2
NKT = S // 128      # 16 k tiles
NQT = SQ // 128     # 4 q tiles per block
NEC = E // 128      # 4 e chunks
NST = S // 128      # 16 s tiles

# row offsets in big16
R_Q, R_K, R_V = 0, 2048, 4096
R_WQ, R_WK, R_WV, R_WO, R_MISC = 6144, 6656, 7168, 7680, 8192
NROW16 = 8194


def split_multi_waits(nc):
    """walrus in this container accepts a single sync-wait command per
    instruction; Tile's tail drain can carry several.  Peel extras onto
    preceding NoOps."""
    def fix_bb(bb):
        insts = list(bb.instructions)
        if not any(i.sync_info and i.sync_info.on_wait and len(i.sync_info.on_wait) > 1
                   for i in insts):
            return
        new = []
        for inst in insts:
            si = inst.sync_info
            if si and si.on_wait and len(si.on_wait) > 1:
                waits = list(si.on_wait)
                for w in waits[:-1]:
                    new.append(mybir.InstNoOp(
                        name=nc.get_next_instruction_name(),
                        engine=inst.engine,
                        bass_nofuse=True,
                        sync_info=mybir.SyncInfo(on_wait=[w], on_update=[]),
                    ))
                inst.sync_info = mybir.SyncInfo(
                    on_wait=[waits[-1]], on_update=list(si.on_update or []))
            new.append(inst)
        bb.instructions = new

    for f in nc.m.functions:
        for bb in f.blocks:
            fix_bb(bb)


def build(waitfix=True):
    nc = bass.Bass()

    big16 = nc.dram_tensor("big16", [NROW16, E], dt.float16, kind="ExternalInput")
    big8 = nc.dram_tensor("big8", [QB * SQ, S], dt.uint8, kind="ExternalInput")
    aux = nc.dram_tensor("aux", [128, 600], dt.float32r, kind="ExternalInput")
    out_d = nc.dram_tensor("out_d", [QB * SQ, E], dt.float16, kind="ExternalOutput")

    with tile.TileContext(nc) as tc, tc.tile_pool(name="persist", bufs=1) as pp:
        # ---------------- persistent tiles (live across all q blocks) -----
        kTp = [pp.tile([128, S], dt.float16, name=f"kTp{i}", tag=f"kTp{i}") for i in range(4)]
        vaug = [pp.tile([128, H * 65], dt.float16, name=f"va{i}", tag=f"va{i}") for i in range(NST)]
        wqf = pp.tile([128, NEC * E], dt.float16)        # q-proj weights, reused per block
        wo_t = pp.tile([64, 8 * E], dt.float16)
        crow = pp.tile([1, E], dt.float16)
        onesc = pp.tile([1, 128], dt.float16)
        cb = pp.tile([128, 24], dt.float32)              # consts | bqs | bks
        cbr = pp.tile([128, 24], dt.float32r)
        eye = pp.tile([65, H * 8], dt.float32r)          # row 64: unit selectors
        sel = pp.tile([8, H * 64], dt.float32r)          # head -> 64-row bcast

        nc.scalar.dma_start(cbr[:], aux[:, 0:24])
        with nc.allow_low_precision(reason="bit-identical f32r->f32 view"):
            nc.vector.tensor_copy(cb[:], cbr[:])
        nc.scalar.dma_start(sel[:], aux[0:8, 24:24 + H * 64])
        nc.scalar.dma_start(eye[64:65, :], aux[8:9, 536:536 + H * 8])
        nc.scalar.dma_start(
            wo_t[:].rearrange("p (c e) -> p c e", c=8),
            big16[R_WO:R_WO + E].rearrange("(c p) e -> p c e", p=64))
        nc.scalar.dma_start(crow[:], big16[R_MISC:R_MISC + 1])
        nc.scalar.dma_start(onesc[:], big16[R_MISC + 1:R_MISC + 2, 0:128])
        nc.scalar.dma_start(wqf[:].rearrange("p (ec e) -> p ec e", ec=NEC),
                            big16[R_WQ:R_WQ + E].rearrange("(ec p) e -> p ec e", p=128))
        for st in range(NST):
            nc.gpsimd.memset(
                vaug[st][:].rearrange("p (h x) -> p h x", h=H)[:, :, 64:65], 1.0)

        # ---------------- k/v load, transpose, projection (once) ----------
        with tc.tile_pool(name="stgw", bufs=1) as stgw, \
             tc.tile_pool(name="stg2", bufs=1) as stg2, \
             tc.tile_pool(name="kv_ps", bufs=4, space="PSUM") as kvp:
            wkf = stgw.tile([128, NEC * E], dt.float16, tag="wkf")
            wvf = stgw.tile([128, NEC * E], dt.float16, tag="wvf")
            for wt, r0 in ((wkf, R_WK), (wvf, R_WV)):
                nc.scalar.dma_start(wt[:].rearrange("p (ec e) -> p ec e", ec=NEC),
                                    big16[r0:r0 + E].rearrange("(ec p) e -> p ec e", p=128))
            for half in range(2):
                kTin = stg2.tile([128, NEC * 1024], dt.float16, tag="kTin")
                vTin = stg2.tile([128, NEC * 1024], dt.float16, tag="vTin")
                for st8 in range(8):
                    r0 = half * 1024 + st8 * 128
                    nc.sync.dma_start(
                        kTin[:].rearrange("p (ec s) -> p ec s", ec=NEC)[:, :, st8 * 128:(st8 + 1) * 128],
                        big16[R_K + r0:R_K + r0 + 128], transpose=True)
                    nc.sync.dma_start(
                        vTin[:].rearrange("p (ec s) -> p ec s", ec=NEC)[:, :, st8 * 128:(st8 + 1) * 128],
                        big16[R_V + r0:R_V + r0 + 128], transpose=True)
                # k^T projection for this half (s columns half*1024 ..)
                for hp in range(4):
                    for sc in range(2):
                        ps = kvp.tile([128, 512], dt.float32, tag="pproj")
                        for ec in range(NEC):
                            nc.tensor.matmul(
                                ps[:],
                                wkf[:, ec * E + hp * 128: ec * E + (hp + 1) * 128],
                                kTin[:, ec * 1024 + sc * 512: ec * 1024 + (sc + 1) * 512],
                                start=(ec == 0), stop=(ec == NEC - 1))
                        nc.scalar.activation(
                            kTp[hp][:, half * 1024 + sc * 512: half * 1024 + (sc + 1) * 512],
                            ps[:], Act.Identity, bias=cb[:, 20 + hp:21 + hp])
                # v projection for this half
                for st8 in range(8):
                    st = half * 8 + st8
                    ps = kvp.tile([128, E], dt.float32, tag="pproj")
                    for ec in range(NEC):
                        nc.tensor.matmul(
                            ps[:],
                            vTin[:, ec * 1024 + st8 * 128: ec * 1024 + (st8 + 1) * 128],
                            wvf[:, ec * E:(ec + 1) * E],
                            start=(ec == 0), stop=(ec == NEC - 1))
                    nc.scalar.activation(
                        vaug[st][:].rearrange("p (h x) -> p h x", h=H)[:, :, 0:64],
                        ps[:], Act.Identity)

        # ---------------- per-query-block pipeline ----------------
        with tc.tile_pool(name="blk", bufs=1) as blk, \
             tc.tile_pool(name="stg3", bufs=2) as stg3, \
             tc.tile_pool(name="qstg", bufs=1) as qstg, \
             tc.tile_pool(name="att", bufs=2) as att, \
             tc.tile_pool(name="attp", bufs=3) as attp, \
             tc.tile_pool(name="pmp", bufs=2) as pmp, \
             tc.tile_pool(name="den", bufs=1) as denp, \
             tc.tile_pool(name="fin2", bufs=1) as fin2, \
             tc.tile_pool(name="qk_ps", bufs=2, space="PSUM") as qkp, \
             tc.tile_pool(name="av_ps", bufs=2, space="PSUM") as avp, \
             tc.tile_pool(name="dall_ps", bufs=1, space="PSUM") as dap, \
             tc.tile_pool(name="qp_ps", bufs=1, space="PSUM") as qpp, \
             tc.tile_pool(name="blk_ps", bufs=1, space="PSUM") as bps:
          for qb in range(QB):
            # per-half tiles: the first half's WAR releases early, letting the
            # next block's staging overlap this block's tail heads
            d0T = [blk.tile([128, 8 * SQ], dt.bfloat16, name=f"d0T{i}", tag=f"d0T{i}")
                   for i in range(2)]
            d1T = [blk.tile([128, 8 * SQ], dt.bfloat16, name=f"d1T{i}", tag=f"d1T{i}")
                   for i in range(2)]
            maskT = [blk.tile([128, 8 * SQ], dt.float16, name=f"maskT{i}", tag=f"maskT{i}")
                     for i in range(2)]
            qTp = [blk.tile([128, SQ], dt.float16, name=f"qTp{i}", tag=f"qTp{i}")
                   for i in range(4)]
            oT = [blk.tile([64, SQ], dt.float16, name=f"oT{i}", tag=f"oT{i}")
                  for i in range(H)]

            # packed mask/d0/d1 byte: unpack on DVE -> XBAR
            for qt in range(NQT):
                bb = stg3.tile([128, S], dt.uint8, tag="bb")
                nc.scalar.dma_start(
                    bb[:], big8.rearrange("(qt p) k -> qt p k", p=128)[qb * NQT + qt])
                mfc = stg3.tile([128, S], dt.float16, tag="mfc")
                d0c = stg3.tile([128, S], dt.bfloat16, tag="d0c")
                d1c = stg3.tile([128, S], dt.bfloat16, tag="d1c")
                # bit-extract on DVE (walrus only allows TensorScalar there);
                # the u8->float casts go to gpsimd, which idles otherwise
                for dst, sh, mk in ((mfc, 7, None), (d0c, 3, 15), (d1c, 0, 7)):
                    iu8 = stg3.tile([128, S], dt.uint8, tag="iu8")
                    if mk is None:
                        nc.vector.tensor_scalar(iu8[:], bb[:], sh, None,
                                                Alu.logical_shift_right)
                    elif sh:
                        nc.vector.tensor_scalar(iu8[:], bb[:], sh, mk,
                                                Alu.logical_shift_right,
                                                Alu.bitwise_and)
                    else:
                        nc.vector.tensor_scalar(iu8[:], bb[:], mk, None,
                                                Alu.bitwise_and)
                    nc.gpsimd.tensor_copy(dst[:], iu8[:])
                for hf in range(2):
                    ksl = slice(hf * 1024, (hf + 1) * 1024)
                    nc.sync.dma_start(
                        maskT[hf][:].rearrange("p (kt q) -> p kt q", kt=8)[:, :, qt * 128:(qt + 1) * 128],
                        mfc[:, ksl], transpose=True)
                    nc.sync.dma_start(
                        d0T[hf][:].rearrange("p (kt q) -> p kt q", kt=8)[:, :, qt * 128:(qt + 1) * 128],
                        d0c[:, ksl], transpose=True)
                    nc.sync.dma_start(
                        d1T[hf][:].rearrange("p (kt q) -> p kt q", kt=8)[:, :, qt * 128:(qt + 1) * 128],
                        d1c[:, ksl], transpose=True)

            # query block: load, transpose, project (pre-scaled by 0.125)
            qTin = qstg.tile([128, NEC * SQ], dt.float16, tag="qTin")
            for st in range(NQT):
                r0 = R_Q + qb * SQ + st * 128
                nc.sync.dma_start(
                    qTin[:].rearrange("p (ec q) -> p ec q", ec=NEC)[:, :, st * 128:(st + 1) * 128],
                    big16[r0:r0 + 128], transpose=True)
            for hp in range(4):
                ps = qpp.tile([128, SQ], dt.float32, tag="bq")
                for ec in range(NEC):
                    nc.tensor.matmul(
                        ps[:], wqf[:, ec * E + hp * 128: ec * E + (hp + 1) * 128],
                        qTin[:, ec * SQ:(ec + 1) * SQ],
                        start=(ec == 0), stop=(ec == NEC - 1))
                nc.scalar.activation(qTp[hp][:], ps[:], Act.Identity,
                                     bias=cb[:, 16 + hp:17 + hp], scale=0.125)

            # attention over all heads for this block
            dall = dap.tile([8, SQ], dt.float32, tag="dall")
            for h in range(H):
                hp, hsub = h // 2, h % 2
                qT_h = qTp[hp][hsub * 64:(hsub + 1) * 64, :]
                r_ap = cb[:, h:h + 1]
                s_ap = cb[:, 8 + h:8 + h + 1]
                av = avp.tile([65, SQ], dt.float32, tag="av")
                for hf in range(2):  # half-head granularity for SBUF
                    y = att.tile([128, 8 * SQ], dt.bfloat16, tag="y")
                    nc.vector.scalar_tensor_tensor(
                        y[:], d1T[hf][:], r_ap, d0T[hf][:], Alu.mult, Alu.add)
                    edm = att.tile([128, 8 * SQ], dt.bfloat16, tag="edm")
                    nc.scalar.activation(edm[:], y[:], Act.Exp, scale=s_ap)
                    for g in range(2):  # groups of 4 k-tiles
                        sn = attp.tile([128, 4 * SQ], dt.bfloat16, tag="sn")
                        for i in range(4):
                            kt = hf * 8 + g * 4 + i
                            qk = qkp.tile([128, SQ], dt.float32, tag="qk")
                            nc.tensor.matmul(
                                qk[:], kTp[hp][hsub * 64:(hsub + 1) * 64,
                                               kt * 128:(kt + 1) * 128],
                                qT_h, start=True, stop=True)
                            nc.vector.scalar_tensor_tensor(
                                sn[:, i * SQ:(i + 1) * SQ],
                                edm[:, (g * 4 + i) * SQ:(g * 4 + i + 1) * SQ],
                                1.0, qk[:], Alu.mult, Alu.subtract)
                        pgrp = attp.tile([128, 4 * SQ], dt.float16, tag="pgrp")
                        nc.scalar.activation(pgrp[:], sn[:], Act.Exp, scale=-1.0)
                        for i in range(4):
                            kt = hf * 8 + g * 4 + i
                            pm = pmp.tile([128, SQ], dt.float16, tag="pm")
                            ktl = g * 4 + i
                            nc.gpsimd.tensor_tensor(
                                pm[:], pgrp[:, i * SQ:(i + 1) * SQ],
                                maskT[hf][:, ktl * SQ:(ktl + 1) * SQ], Alu.mult)
                            nc.tensor.matmul(
                                av[:],
                                vaug[kt][:].rearrange("p (hh x) -> p hh x", hh=H)[:, h, :],
                                pm[:], start=(kt == 0), stop=(kt == NKT - 1))
                # attention rows -> per-head sbuf; denominator -> dall row h
                nc.scalar.activation(oT[h][:], av[0:64, :], Act.Identity)
                den = denp.tile([65, SQ], dt.float32r, tag="den")
                nc.vector.tensor_copy(den[64:65, :], av[64:65, :])
                nc.tensor.matmul(dall[:], eye[64:65, h * 8:(h + 1) * 8],
                                 den[64:65, :], start=(h == 0), stop=(h == H - 1))

            # normalize + output projection for this block
            rcp = fin2.tile([8, SQ], dt.float32r, tag="rcp")
            with nc.allow_low_precision(reason="f32r view of f32 reciprocal"):
                nc.vector.reciprocal(rcp[:], dall[:])
            for h in range(H):
                bc = bps.tile([64, SQ], dt.float32, tag="bc")
                nc.tensor.matmul(bc[:], sel[:, h * 64:(h + 1) * 64], rcp[:],
                                 start=True, stop=True)
                nc.vector.scalar_tensor_tensor(oT[h][:], oT[h][:], 1.0, bc[:],
                                               Alu.mult, Alu.mult)
            for st in range(NQT):
                fo = bps.tile([128, E], dt.float32, tag="fo")
                for ec8 in range(8):
                    nc.tensor.matmul(
                        fo[:], oT[ec8][:, st * 128:(st + 1) * 128],
                        wo_t[:, ec8 * E:(ec8 + 1) * E],
                        start=(ec8 == 0), stop=False)
                nc.tensor.matmul(fo[:], onesc[:], crow[:],
                                 start=False, stop=True)
                ot = fin2.tile([128, E], dt.float16, tag="ot")
                nc.scalar.activation(ot[:], fo[:], Act.Identity)
                nc.scalar.dma_start(
                    out_d.rearrange("(st p) e -> st p e", p=128)[qb * NQT + st],
                    ot[:])

    if waitfix:
        split_multi_waits(nc)
    return nc


_cache = {}


def _pack_inputs(query, key, value, mask, dict_mask, wq, bq, wk, bk, wv, bv,
                 wo, bo, head_weights):
    """Build the 3 consolidated per-core input tensors (host-side)."""
    q16 = np.asarray(query, np.float16)
    k16 = np.asarray(key, np.float16)
    v16 = np.asarray(value, np.float16)
    m8 = (np.asarray(mask) != 0).astype(np.uint8)
    dmf = np.asarray(dict_mask, np.float32)
    d0q = np.rint(dmf[0] * 15.0).astype(np.uint8)
    d1q = np.rint(dmf[1] * 7.0).astype(np.uint8)
    packed = ((m8 << 7) | (d0q << 3) | d1q).astype(np.uint8)
    wq16 = np.asarray(wq, np.float16)
    wk16 = np.asarray(wk, np.float16)
    wv16 = np.asarray(wv, np.float16)
    wo32 = np.asarray(wo, np.float32)
    bq = np.asarray(bq, np.float32)
    bk = np.asarray(bk, np.float32)
    bv = np.asarray(bv, np.float32)
    bo = np.asarray(bo, np.float32)
    hw = np.asarray(head_weights, np.float32)

    # dm_h = s_h * (d0q + r_h * d1q) with s_h = a/15, r_h = 15*b/(7*a)
    aux = np.zeros((128, 600), np.float32)
    for h in range(H):
        a, b_ = float(hw[h, 0]), float(hw[h, 1])
        if abs(a) < 1e-20:
            a = 1e-20 if a >= 0 else -1e-20
        aux[:, h] = 15.0 * b_ / (7.0 * a)
        aux[:, 8 + h] = a / 15.0
    aux[:, 16:20] = 0.125 * bq.reshape(NEC, 128).T
    aux[:, 20:24] = bk.reshape(NEC, 128).T
    for h in range(H):
        aux[h, 24 + h * 64:24 + (h + 1) * 64] = 1.0
        aux[8, 536 + h * 8 + h] = 1.0

    crow = (bv @ wo32 + bo).astype(np.float16)

    in_maps = []
    for c in range(NCORE):
        b = c
        b16 = np.empty((NROW16, E), np.float16)
        b16[R_Q:R_Q + S] = q16[b]
        b16[R_K:R_K + S] = k16[b]
        b16[R_V:R_V + S] = v16[b]
        b16[R_WQ:R_WQ + E] = wq16
        b16[R_WK:R_WK + E] = wk16
        b16[R_WV:R_WV + E] = wv16
        b16[R_WO:R_WO + E] = np.asarray(wo32, np.float16)
        b16[R_MISC, :] = crow
        b16[R_MISC + 1, :] = 0
        b16[R_MISC + 1, 0:128] = 1.0
        in_maps.append({"big16": b16,
                        "big8": np.ascontiguousarray(packed[b]),
                        "aux": aux})
    return in_maps


def kernel(query, key, value, mask, dict_mask, wq, bq, wk, bk, wv, bv, wo, bo,
           head_weights):
    if "nc" not in _cache:
        _cache["nc"] = build()
    nc = _cache["nc"]

    in_maps = _pack_inputs(query, key, value, mask, dict_mask, wq, bq, wk, bk,
                           wv, bv, wo, bo, head_weights)
    res = run_bass_kernel_spmd(nc, in_maps, core_ids=list(range(NCORE)))
    out = np.empty((B, S, E), np.float32)
    for c in range(NCORE):
        out[c] = res.results[c]["out_d"].astype(np.float32)
    return out


def make_in_maps(inputs):
    """Rebuild the per-core input maps from the full input dict (test helper)."""
    if "nc" not in _cache:
        _cache["nc"] = build()
    return _pack_inputs(
        inputs["query"], inputs["key"], inputs["value"], inputs["mask"],
        inputs["dict_mask"], inputs["wq"], inputs["bq"], inputs["wk"],
        inputs["bk"], inputs["wv"], inputs["bv"], inputs["wo"], inputs["bo"],
        inputs["head_weights"])


# revision 52
# speedup vs baseline: 1.0404x; 1.0404x over previous
"""Bass/Tile kernel for nn_MultiHeadAttention_84104049590613 on trn2.

Sharding: 2 cores, core c handles batch b = c (all 2048 query rows, all 8
heads), looping over 4 query blocks of 512 rows.  K/V and the weights are
loaded and projected once per core (no cross-core duplication at all); the
S^2-sized mask/dict_mask tensors shard perfectly along (b, q).  This layout
minimizes total host->device bytes per invocation, which dominates the
per-call cost of the 8-terminal axon runtime (dispatch overhead is flat in
core count; staging contention grows with active streams).

Host-side input compression/consolidation (3 input tensors, 1 output):
 - big16 [8194, 512] f16: q | k | v | wq | wk | wv | wo | crow/ones rows
 - big8 [2048, 2048] u8: one byte per (q,k) = mask<<7 | round(d0*15)<<3 |
   round(d1*7)  (the quantization rescales fold into the per-head exp
   scale/ratio consts; end-to-end quantization error is <1e-3)
 - aux [128, 600] f32r: per-head ratio/scale consts | q/k biases | the
   constant 0/1 selector matrices
Output is fp16, upcast on host.

Score layout on chip is transposed: [k (partition), q (free)] so that the
attention*V matmul contracts k on the partition dim directly and the softmax
denominator comes free as a ones-column appended to V.  Softmax uses no
max-subtraction (scores are O(5); exp never overflows) and the 0/1 mask is
applied multiplicatively after exp.
"""

import numpy as np

import concourse.bass as bass
import concourse.mybir as mybir
import concourse.tile as tile
from concourse.bass_utils import run_bass_kernel_spmd

dt = mybir.dt
Alu = mybir.AluOpType
Act = mybir.ActivationFunctionType

B, S, E, H, DH = 2, 2048, 512, 8, 64
SQ = 512            # query rows per block
QB = 4              # query blocks per core
NCORE = 2
NKT = S // 128      # 16 k tiles
NQT = SQ // 128     # 4 q tiles per block
NEC = E // 128      # 4 e chunks
NST = S // 128      # 16 s tiles

# row offsets in big16
R_Q, R_K, R_V = 0, 2048, 4096
R_WQ, R_WK, R_WV, R_WO, R_MISC = 6144, 6656, 7168, 7680, 8192
NROW16 = 8194


def split_multi_waits(nc):
    """walrus in this container accepts a single sync-wait command per
    instruction; Tile's tail drain can carry several.  Peel extras onto
    preceding NoOps."""
    def fix_bb(bb):
        insts = list(bb.instructions)
        if not any(i.sync_info and i.sync_info.on_wait and len(i.sync_info.on_wait) > 1
                   for i in insts):
            return
        new = []
        for inst in insts:
            si = inst.sync_info
            if si and si.on_wait and len(si.on_wait) > 1:
                waits = list(si.on_wait)
                for w in waits[:-1]:
                    new.append(mybir.InstNoOp(
                        name=nc.get_next_instruction_name(),
                        engine=inst.engine,
                        bass_nofuse=True,
                        sync_info=mybir.SyncInfo(on_wait=[w], on_update=[]),
                    ))
                inst.sync_info = mybir.SyncInfo(
                    on_wait=[waits[-1]], on_update=list(si.on_update or []))
            new.append(inst)
        bb.instructions = new

    for f in nc.m.functions:
        for bb in f.blocks:
            fix_bb(bb)


def build(waitfix=True):
    nc = bass.Bass()

    big16 = nc.dram_tensor("big16", [NROW16, E], dt.float16, kind="ExternalInput")
    big8 = nc.dram_tensor("big8", [QB * SQ, S], dt.uint8, kind="ExternalInput")
    aux = nc.dram_tensor("aux", [128, 600], dt.float32r, kind="ExternalInput")
    out_d = nc.dram_tensor("out_d", [QB * SQ, E], dt.float16, kind="ExternalOutput")

    with tile.TileContext(nc) as tc, tc.tile_pool(name="persist", bufs=1) as pp:
        # ---------------- persistent tiles (live across all q blocks) -----
        kTp = [pp.tile([128, S], dt.float16, name=f"kTp{i}", tag=f"kTp{i}") for i in range(4)]
        vaug = [pp.tile([128, H * 65], dt.float16, name=f"va{i}", tag=f"va{i}") for i in range(NST)]
        wqf = pp.tile([128, NEC * E], dt.float16)        # q-proj weights, reused per block
        wo_t = pp.tile([64, 8 * E], dt.float16)
        crow = pp.tile([1, E], dt.float16)
        onesc = pp.tile([1, 128], dt.float16)
        cb = pp.tile([128, 24], dt.float32)              # consts | bqs | bks
        cbr = pp.tile([128, 24], dt.float32r)
        eye = pp.tile([65, H * 8], dt.float32r)          # row 64: unit selectors
        sel = pp.tile([8, H * 64], dt.float32r)          # head -> 64-row bcast

        nc.scalar.dma_start(cbr[:], aux[:, 0:24])
        with nc.allow_low_precision(reason="bit-identical f32r->f32 view"):
            nc.vector.tensor_copy(cb[:], cbr[:])
        nc.scalar.dma_start(sel[:], aux[0:8, 24:24 + H * 64])
        nc.scalar.dma_start(eye[64:65, :], aux[8:9, 536:536 + H * 8])
        nc.scalar.dma_start(
            wo_t[:].rearrange("p (c e) -> p c e", c=8),
            big16[R_WO:R_WO + E].rearrange("(c p) e -> p c e", p=64))
        nc.scalar.dma_start(crow[:], big16[R_MISC:R_MISC + 1])
        nc.scalar.dma_start(onesc[:], big16[R_MISC + 1:R_MISC + 2, 0:128])
        nc.scalar.dma_start(wqf[:].rearrange("p (ec e) -> p ec e", ec=NEC),
                            big16[R_WQ:R_WQ + E].rearrange("(ec p) e -> p ec e", p=128))
        for st in range(NST):
            nc.gpsimd.memset(
                vaug[st][:].rearrange("p (h x) -> p h x", h=H)[:, :, 64:65], 1.0)

        # ---------------- k/v load, transpose, projection (once) ----------
        with tc.tile_pool(name="stgw", bufs=1) as stgw, \
             tc.tile_pool(name="stg2", bufs=1) as stg2, \
             tc.tile_pool(name="kv_ps", bufs=4, space="PSUM") as kvp:
            wkf = stgw.tile([128, NEC * E], dt.float16, tag="wkf")
            wvf = stgw.tile([128, NEC * E], dt.float16, tag="wvf")
            for wt, r0 in ((wkf, R_WK), (wvf, R_WV)):
                nc.scalar.dma_start(wt[:].rearrange("p (ec e) -> p ec e", ec=NEC),
                                    big16[r0:r0 + E].rearrange("(ec p) e -> p ec e", p=128))
            for half in range(2):
                kTin = stg2.tile([128, NEC * 1024], dt.float16, tag="kTin")
                vTin = stg2.tile([128, NEC * 1024], dt.float16, tag="vTin")
                for st8 in range(8):
                    r0 = half * 1024 + st8 * 128
                    nc.sync.dma_start(
                        kTin[:].rearrange("p (ec s) -> p ec s", ec=NEC)[:, :, st8 * 128:(st8 + 1) * 128],
                        big16[R_K + r0:R_K + r0 + 128], transpose=True)
                    nc.sync.dma_start(
                        vTin[:].rearrange("p (ec s) -> p ec s", ec=NEC)[:, :, st8 * 128:(st8 + 1) * 128],
                        big16[R_V + r0:R_V + r0 + 128], transpose=True)
                # k^T projection for this half (s columns half*1024 ..)
                for hp in range(4):
                    for sc in range(2):
                        ps = kvp.tile([128, 512], dt.float32, tag="pproj")
                        for ec in range(NEC):
                            nc.tensor.matmul(
                                ps[:],
                                wkf[:, ec * E + hp * 128: ec * E + (hp + 1) * 128],
                                kTin[:, ec * 1024 + sc * 512: ec * 1024 + (sc + 1) * 512],
                                start=(ec == 0), stop=(ec == NEC - 1))
                        nc.scalar.activation(
                            kTp[hp][:, half * 1024 + sc * 512: half * 1024 + (sc + 1) * 512],
                            ps[:], Act.Identity, bias=cb[:, 20 + hp:21 + hp])
                # v projection for this half
                for st8 in range(8):
                    st = half * 8 + st8
                    ps = kvp.tile([128, E], dt.float32, tag="pproj")
                    for ec in range(NEC):
                        nc.tensor.matmul(
                            ps[:],
                            vTin[:, ec * 1024 + st8 * 128: ec * 1024 + (st8 + 1) * 128],
                            wvf[:, ec * E:(ec + 1) * E],
                            start=(ec == 0), stop=(ec == NEC - 1))
                    nc.scalar.activation(
                        vaug[st][:].rearrange("p (h x) -> p h x", h=H)[:, :, 0:64],
                        ps[:], Act.Identity)

        # ---------------- per-query-block pipeline ----------------
        with tc.tile_pool(name="blk", bufs=1) as blk, \
             tc.tile_pool(name="stg3", bufs=2) as stg3, \
             tc.tile_pool(name="qstg", bufs=1) as qstg, \
             tc.tile_pool(name="att", bufs=2) as att, \
             tc.tile_pool(name="attp", bufs=3) as attp, \
             tc.tile_pool(name="pmp", bufs=2) as pmp, \
             tc.tile_pool(name="den", bufs=1) as denp, \
             tc.tile_pool(name="fin2", bufs=1) as fin2, \
             tc.tile_pool(name="qk_ps", bufs=2, space="PSUM") as qkp, \
             tc.tile_pool(name="av_ps", bufs=2, space="PSUM") as avp, \
             tc.tile_pool(name="dall_ps", bufs=1, space="PSUM") as dap, \
             tc.tile_pool(name="qp_ps", bufs=1, space="PSUM") as qpp, \
             tc.tile_pool(name="blk_ps", bufs=1, space="PSUM") as bps:
          for qb in range(QB):
            # per-half tiles: the first half's WAR releases early, letting the
            # next block's staging overlap this block's tail heads
            d0T = [blk.tile([128, 8 * SQ], dt.bfloat16, name=f"d0T{i}", tag=f"d0T{i}")
                   for i in range(2)]
            d1T = [blk.tile([128, 8 * SQ], dt.bfloat16, name=f"d1T{i}", tag=f"d1T{i}")
                   for i in range(2)]
            maskT = [blk.tile([128, 8 * SQ], dt.float16, name=f"maskT{i}", tag=f"maskT{i}")
                     for i in range(2)]
            qTp = [blk.tile([128, SQ], dt.float16, name=f"qTp{i}", tag=f"qTp{i}")
                   for i in range(4)]
            oT = [blk.tile([64, SQ], dt.float16, name=f"oT{i}", tag=f"oT{i}")
                  for i in range(H)]

            # packed mask/d0/d1 byte: unpack on DVE -> XBAR
            for qt in range(NQT):
                bb = stg3.tile([128, S], dt.uint8, tag="bb")
                nc.scalar.dma_start(
                    bb[:], big8.rearrange("(qt p) k -> qt p k", p=128)[qb * NQT + qt])
                mfc = stg3.tile([128, S], dt.float16, tag="mfc")
                d0c = stg3.tile([128, S], dt.bfloat16, tag="d0c")
                d1c = stg3.tile([128, S], dt.bfloat16, tag="d1c")
                # bit-extract on DVE (walrus only allows TensorScalar there);
                # the u8->float casts go to gpsimd, which idles otherwise
                for dst, sh, mk in ((mfc, 7, None), (d0c, 3, 15), (d1c, 0, 7)):
                    iu8 = stg3.tile([128, S], dt.uint8, tag="iu8")
                    if mk is None:
                        nc.vector.tensor_scalar(iu8[:], bb[:], sh, None,
                                                Alu.logical_shift_right)
                    elif sh:
                        nc.vector.tensor_scalar(iu8[:], bb[:], sh, mk,
                                                Alu.logical_shift_right,
                                                Alu.bitwise_and)
                    else:
                        nc.vector.tensor_scalar(iu8[:], bb[:], mk, None,
                                                Alu.bitwise_and)
                    nc.gpsimd.tensor_copy(dst[:], iu8[:])
                for hf in range(2):
                    ksl = slice(hf * 1024, (hf + 1) * 1024)
                    nc.sync.dma_start(
                        maskT[hf][:].rearrange("p (kt q) -> p kt q", kt=8)[:, :, qt * 128:(qt + 1) * 128],
                        mfc[:, ksl], transpose=True)
                    nc.sync.dma_start(
                        d0T[hf][:].rearrange("p (kt q) -> p kt q", kt=8)[:, :, qt * 128:(qt + 1) * 128],
                        d0c[:, ksl], transpose=True)
                    nc.sync.dma_start(
                        d1T[hf][:].rearrange("p (kt q) -> p kt q", kt=8)[:, :, qt * 128:(qt + 1) * 128],
                        d1c[:, ksl], transpose=True)

            # query block: load, transpose, project (pre-scaled by 0.125)
            qTin = qstg.tile([128, NEC * SQ], dt.float16, tag="qTin")
            for st in range(NQT):
                r0 = R_Q + qb * SQ + st * 128
                nc.sync.dma_start(
                    qTin[:].rearrange("p (ec q) -> p ec q", ec=NEC)[:, :, st * 128:(st + 1) * 128],
                    big16[r0:r0 + 128], transpose=True)
            for hp in range(4):
                ps = qpp.tile([128, SQ], dt.float32, tag="bq")
                for ec in range(NEC):
                    nc.tensor.matmul(
                        ps[:], wqf[:, ec * E + hp * 128: ec * E + (hp + 1) * 128],
                        qTin[:, ec * SQ:(ec + 1) * SQ],
                        start=(ec == 0), stop=(ec == NEC - 1))
                nc.scalar.activation(qTp[hp][:], ps[:], Act.Identity,
                                     bias=cb[:, 16 + hp:17 + hp], scale=0.125)

            # attention over all heads for this block
            dall = dap.tile([8, SQ], dt.float32, tag="dall")
            for h in range(H):
                hp, hsub = h // 2, h % 2
                qT_h = qTp[hp][hsub * 64:(hsub + 1) * 64, :]
                r_ap = cb[:, h:h + 1]
                s_ap = cb[:, 8 + h:8 + h + 1]
                av = avp.tile([65, SQ], dt.float32, tag="av")
                for hf in range(2):  # half-head granularity for SBUF
                    y = att.tile([128, 8 * SQ], dt.bfloat16, tag="y")
                    nc.vector.scalar_tensor_tensor(
                        y[:], d1T[hf][:], r_ap, d0T[hf][:], Alu.mult, Alu.add)
                    edm = att.tile([128, 8 * SQ], dt.bfloat16, tag="edm")
                    nc.scalar.activation(edm[:], y[:], Act.Exp, scale=s_ap)
                    for g in range(2):  # groups of 4 k-tiles
                        sn = attp.tile([128, 4 * SQ], dt.bfloat16, tag="sn")
                        for i in range(4):
                            kt = hf * 8 + g * 4 + i
                            qk = qkp.tile([128, SQ], dt.float32, tag="qk")
                            nc.tensor.matmul(
                                qk[:], kTp[hp][hsub * 64:(hsub + 1) * 64,
                                               kt * 128:(kt + 1) * 128],
                                qT_h, start=True, stop=True)
                            nc.vector.scalar_tensor_tensor(
                                sn[:, i * SQ:(i + 1) * SQ],
                                edm[:, (g * 4 + i) * SQ:(g * 4 + i + 1) * SQ],
                                1.0, qk[:], Alu.mult, Alu.subtract)
                        pgrp = attp.tile([128, 4 * SQ], dt.float16, tag="pgrp")
                        nc.scalar.activation(pgrp[:], sn[:], Act.Exp, scale=-1.0)
                        for i in range(4):
                            kt = hf * 8 + g * 4 + i
                            pm = pmp.tile([128, SQ], dt.float16, tag="pm")
                            ktl = g * 4 + i
                            nc.gpsimd.tensor_tensor(
                                pm[:], pgrp[:, i * SQ:(i + 1) * SQ],
                                maskT[hf][:, ktl * SQ:(ktl + 1) * SQ], Alu.mult)
                            nc.tensor.matmul(
                                av[:],
                                vaug[kt][:].rearrange("p (hh x) -> p hh x", hh=H)[:, h, :],
                                pm[:], start=(kt == 0), stop=(kt == NKT - 1))
                # attention rows -> per-head sbuf; denominator -> dall row h
                nc.scalar.activation(oT[h][:], av[0:64, :], Act.Identity)
                den = denp.tile([65, SQ], dt.float32r, tag="den")
                nc.vector.tensor_copy(den[64:65, :], av[64:65, :])
                nc.tensor.matmul(dall[:], eye[64:65, h * 8:(h + 1) * 8],
                                 den[64:65, :], start=(h == 0), stop=(h == H - 1))

            # normalize + output projection for this block
            rcp = fin2.tile([8, SQ], dt.float32r, tag="rcp")
            with nc.allow_low_precision(reason="f32r view of f32 reciprocal"):
                nc.vector.reciprocal(rcp[:], dall[:])
            for h in range(H):
                bc = bps.tile([64, SQ], dt.float32, tag="bc")
                nc.tensor.matmul(bc[:], sel[:, h * 64:(h + 1) * 64], rcp[:],
                                 start=True, stop=True)
                nc.vector.scalar_tensor_tensor(oT[h][:], oT[h][:], 1.0, bc[:],
                                               Alu.mult, Alu.mult)
            for st in range(NQT):
                fo = bps.tile([128, E], dt.float32, tag="fo")
                for ec8 in range(8):
                    nc.tensor.matmul(
                        fo[:], oT[ec8][:, st * 128:(st + 1) * 128],
                        wo_t[:, ec8 * E:(ec8 + 1) * E],
                        start=(ec8 == 0), stop=False)
                nc.tensor.matmul(fo[:], onesc[:], crow[:],
                                 start=False, stop=True)
                ot = fin2.tile([128, E], dt.float16, tag="ot")
                nc.scalar.activation(ot[:], fo[:], Act.Identity)
                nc.scalar.dma_start(
                    out_d.rearrange("(st p) e -> st p e", p=128)[qb * NQT + st],
                    ot[:])

    if waitfix:
        split_multi_waits(nc)
    return nc


_cache = {}


def _pack_inputs(query, key, value, mask, dict_mask, wq, bq, wk, bk, wv, bv,
                 wo, bo, head_weights):
    """Build the 3 consolidated per-core input tensors (host-side)."""
    q16 = np.asarray(query, np.float16)
    k16 = np.asarray(key, np.float16)
    v16 = np.asarray(value, np.float16)
    m8 = (np.asarray(mask) != 0).astype(np.uint8)
    dmf = np.asarray(dict_mask, np.float32)
    d0q = np.rint(dmf[0] * 15.0).astype(np.uint8)
    d1q = np.rint(dmf[1] * 7.0).astype(np.uint8)
    packed = ((m8 << 7) | (d0q << 3) | d1q).astype(np.uint8)
    wq16 = np.asarray(wq, np.float16)
    wk16 = np.asarray(wk, np.float16)
    wv16 = np.asarray(wv, np.float16)
    wo32 = np.asarray(wo, np.float32)
    bq = np.asarray(bq, np.float32)
    bk = np.asarray(bk, np.float32)
    bv = np.asarray(bv, np.float32)
    bo = np.asarray(bo, np.float32)
    hw = np.asarray(head_weights, np.float32)

    # dm_h = s_h * (d0q + r_h * d1q) with s_h = a/15, r_h = 15*b/(7*a)
    aux = np.zeros((128, 600), np.float32)
    for h in range(H):
        a, b_ = float(hw[h, 0]), float(hw[h, 1])
        if abs(a) < 1e-20:
            a = 1e-20 if a >= 0 else -1e-20
        aux[:, h] = 15.0 * b_ / (7.0 * a)
        aux[:, 8 + h] = a / 15.0
    aux[:, 16:20] = 0.125 * bq.reshape(NEC, 128).T
    aux[:, 20:24] = bk.reshape(NEC, 128).T
    for h in range(H):
        aux[h, 24 + h * 64:24 + (h + 1) * 64] = 1.0
        aux[8, 536 + h * 8 + h] = 1.0

    crow = (bv @ wo32 + bo).astype(np.float16)

    in_maps = []
    for c in range(NCORE):
        b = c
        b16 = np.empty((NROW16, E), np.float16)
        b16[R_Q:R_Q + S] = q16[b]
        b16[R_K:R_K + S] = k16[b]
        b16[R_V:R_V + S] = v16[b]
        b16[R_WQ:R_WQ + E] = wq16
        b16[R_WK:R_WK + E] = wk16
        b16[R_WV:R_WV + E] = wv16
        b16[R_WO:R_WO + E] = np.asarray(wo32, np.float16)
        b16[R_MISC, :] = crow
        b16[R_MISC + 1, :] = 0
        b16[R_MISC + 1, 0:128] = 1.0
        in_maps.append({"big16": b16,
                        "big8": np.ascontiguousarray(packed[b]),
                        "aux": aux})
    return in_maps


def kernel(query, key, value, mask, dict_mask, wq, bq, wk, bk, wv, bv, wo, bo,
           head_weights):
    if "nc" not in _cache:
        _cache["nc"] = build()
    nc = _cache["nc"]

    in_maps = _pack_inputs(query, key, value, mask, dict_mask, wq, bq, wk, bk,
                           wv, bv, wo, bo, head_weights)
    res = run_bass_kernel_spmd(nc, in_maps, core_ids=list(range(NCORE)))
    out = np.empty((B, S, E), np.float32)
    for c in range(NCORE):
        out[c] = res.results[c]["out_d"].astype(np.float32)
    return out


def make_in_maps(inputs):
    """Rebuild the per-core input maps from the full input dict (test helper)."""
    if "nc" not in _cache:
        _cache["nc"] = build()
    return _pack_inputs(
        inputs["query"], inputs["key"], inputs["value"], inputs["mask"],
        inputs["dict_mask"], inputs["wq"], inputs["bq"], inputs["wk"],
        inputs["bk"], inputs["wv"], inputs["bv"], inputs["wo"], inputs["bo"],
        inputs["head_weights"])
